# revision 10
# baseline (speedup 1.0000x reference)
"""Log-normal NLL loss kernel for Trainium2 (8 NeuronCores, data-parallel).

Reference math (per sample):
    preds = (mu1, log_sigma1, mu2, log_sigma2); y = truth
    s   = exp(2*log_sigma1) + exp(2*log_sigma2)          # sigma1^2 + sigma2^2
    mu  = mu1 + mu2
    out = log(y) + 0.5*log(2*pi*s) + (log(y) - mu)^2 / (2*s)

The division by s is computed on the Scalar (ACT) engine via exp/ln
(v = exp(ln(z^2/2) - ln(2*pi*s) + ln(2*pi))), which avoids the slow DVE
iterative-divide and balances work: 7 ACT passes + 6 DVE passes per
element, both under the HBM roofline (~24 MB/core @ ~358 GB/s ~= 67 us).
"""

import os
import sys

import numpy as np

for _p in ("/opt/trn_rl_repo", os.path.expanduser("~/.axon_site/_ro/trn_rl_repo")):
    if os.path.isdir(_p) and _p not in sys.path:
        sys.path.insert(0, _p)

import concourse.bacc as bacc
import concourse.bass as bass
import concourse.mybir as mybir
import concourse.tile as tile
from concourse.bass_utils import run_bass_kernel_spmd

B = 8388608
N_CORES = 8
BC = B // N_CORES  # 1048576 samples per core
P = 128            # SBUF partitions

LOG_2PI = float(np.log(2.0 * np.pi))
TWO_PI = float(2.0 * np.pi)
INV_SQRT2 = float(1.0 / np.sqrt(2.0))

_ACT_SET = "natural_log_exp_and_others"  # contains exp, ln AND square
_tables_patched = False


def _pin_act_table_set():
    """Pin all activations to one table set.

    The stock insert_act_table_loads maps each function to the FIRST set
    containing it (exp/square -> exp_and_others, ln -> natural_log_exp...),
    which forces a ~1.3us ACT_TABLE_LOAD at nearly every function switch
    (21 loads, ~27us of Scalar-engine time for this kernel).  Exp, Ln and
    Square all live in natural_log_exp_and_others, so present every other
    set as empty; indices (act_func_set_id) are preserved so walrus still
    resolves the right table.
    """
    global _tables_patched
    if _tables_patched:
        return
    import concourse.hw_specs as hw_specs

    real_fn = hw_specs.get_activation_tables

    def pinned(module_arch):
        real = real_fn(module_arch)
        assert _ACT_SET in real, sorted(real)
        return {
            name: (fns if name == _ACT_SET else set())
            for name, fns in real.items()
        }

    bacc.get_activation_tables = pinned
    _tables_patched = True


def _tile_sizes(r: int, f: int) -> list[int]:
    """Tile sizes summing to r: a few small warmup tiles to prime the
    pipeline quickly, then full-size `f` tiles."""
    warmup = [f // 4, f // 4, f // 2]
    if r <= sum(warmup) or r % f != 0:
        # fall back to uniform tiles
        g = min(f, r)
        while r % g:
            g //= 2
        return [g] * (r // g)
    rest = r - sum(warmup)
    return warmup + [f] * (rest // f) if rest % f == 0 else [f] * (r // f)


def build_nc(bc: int = BC, f: int = 2048) -> bass.Bass:
    """Build the per-core Bass program for a shard of `bc` samples.

    `f` = samples per partition per full tile. Requires bc % (P*f) == 0.
    """
    r = bc // P           # samples per partition
    assert bc % P == 0 and r % f == 0, (bc, f)
    sizes = _tile_sizes(r, f)
    assert sum(sizes) == r

    f32 = mybir.dt.float32
    bf16 = mybir.dt.bfloat16
    Exp = mybir.ActivationFunctionType.Exp
    Ln = mybir.ActivationFunctionType.Ln
    Square = mybir.ActivationFunctionType.Square
    Alu = mybir.AluOpType

    _pin_act_table_set()
    nc = bacc.Bacc()

    # The Exp bias below needs a per-partition const AP; only 0.0/1.0 are
    # pre-registered, so register ln(pi) the same way Bass.__init__ does.
    LN_PI = float(np.log(np.pi))
    bias_t = nc.alloc_sbuf_tensor("const-lnpi", [128, 1], f32)
    nc.gpsimd.memset(bias_t.ap(), LN_PI)
    nc.const_aps.aps[(f32, LN_PI)] = bias_t.ap()
    nc.all_engine_barrier()

    preds = nc.dram_tensor("preds", [bc, 4], f32, kind="ExternalInput")
    truth = nc.dram_tensor("truth", [bc, 1], f32, kind="ExternalInput")
    loss = nc.dram_tensor("loss", [bc], f32, kind="ExternalOutput")

    # Partition p owns the contiguous sample range [p*r, (p+1)*r) so every
    # DMA moves one contiguous chunk per partition.
    preds_v = preds[:].rearrange("(p r) c -> p r c", p=P)   # [128, r, 4]
    truth_v = truth[:].rearrange("(p r) c -> p (r c)", p=P)  # [128, r]
    loss_v = loss[:].rearrange("(p r) -> p r", p=P)          # [128, r]

    nt = len(sizes)
    offs = [sum(sizes[:i]) for i in range(nt)]

    with tile.TileContext(nc) as tc:
        with (
            tc.tile_pool(name="io", bufs=2) as io_pool,
            tc.tile_pool(name="tmp", bufs=2) as tmp,
        ):
            state: dict[int, dict] = {}

            def emit_load(i):
                ftile = sizes[i]
                sl = slice(offs[i], offs[i] + ftile)
                pt = io_pool.tile([P, f, 4], f32, tag="preds", name="pt")[
                    :, :ftile, :
                ]
                nc.sync.dma_start(out=pt[:], in_=preds_v[:, sl, :])
                yt = io_pool.tile([P, f], f32, tag="truth", name="yt")[:, :ftile]
                nc.sync.dma_start(out=yt[:], in_=truth_v[:, sl])
                st = {"pt": pt, "yt": yt}
                for tag in ("ca", "cb", "cc", "cd", "ce"):
                    st[tag] = tmp.tile([P, f], f32, tag=tag, name=tag)[:, :ftile]
                state[i] = st

            def emit_A(i):  # ACT: depends only on tile i's DMAs
                st = state[i]
                pt, yt = st["pt"], st["yt"]
                nc.scalar.activation(st["ca"][:], pt[:, :, 1], Exp, scale=2.0)
                nc.scalar.activation(st["cb"][:], pt[:, :, 3], Exp, scale=2.0)
                nc.scalar.activation(st["cd"][:], yt[:], Ln)       # cd = logy

            def emit_B(i):  # DVE: s, mu, z
                st = state[i]
                pt = st["pt"]
                nc.vector.tensor_add(st["cc"][:], st["ca"][:], st["cb"][:])  # s
                nc.vector.tensor_add(st["ca"][:], pt[:, :, 0], pt[:, :, 2])  # mu
                nc.vector.tensor_sub(st["cb"][:], st["cd"][:], st["ca"][:])  # z

            def emit_C(i):  # ACT: z^2, ln(2pi s), 1/(2s)
                st = state[i]
                nc.scalar.activation(st["ca"][:], st["cb"][:], Square)
                nc.scalar.activation(st["ce"][:], st["cc"][:], Ln, scale=TWO_PI)
                nc.scalar.activation(
                    st["cc"][:], st["ce"][:], Exp, scale=-1.0, bias=LN_PI
                )

            def emit_D(i):  # DVE: w, v, loss; then store
                st = state[i]
                ftile = sizes[i]
                sl = slice(offs[i], offs[i] + ftile)
                yt = st["yt"]
                ot = io_pool.tile([P, f], f32, tag="ot", name="ot")[:, :ftile]
                nc.vector.scalar_tensor_tensor(
                    yt[:], st["ce"][:], 0.5, st["cd"][:], Alu.mult, Alu.add
                )  # yt = 0.5*ln(2pi*s) + logy
                nc.vector.tensor_mul(st["cb"][:], st["ca"][:], st["cc"][:])
                nc.vector.tensor_add(ot[:], yt[:], st["cb"][:])
                nc.sync.dma_start(out=loss_v[:, sl], in_=ot[:])
                del state[i]

            # Software-pipelined emission: tile i's late stages (C: ACT,
            # D: DVE) are emitted AFTER tile i+1's early stages, so each
            # engine's static instruction stream always has ready work
            # instead of blocking on the other engine mid-tile.
            emit_load(0)
            if nt > 1:
                emit_load(1)
            for i in range(nt):
                emit_A(i)
                emit_B(i)
                if i + 2 < nt:
                    emit_load(i + 2)
                if i > 0:
                    emit_C(i - 1)
                    emit_D(i - 1)
            emit_C(nt - 1)
            emit_D(nt - 1)

    nc.compile()
    return nc


_NC = None


def _get_nc() -> bass.Bass:
    global _NC
    if _NC is None:
        _NC = build_nc()
    return _NC


def kernel(preds: np.ndarray, truth: np.ndarray) -> np.ndarray:
    assert preds.shape == (B, 4) and truth.shape == (B, 1)
    nc = _get_nc()
    preds = np.ascontiguousarray(preds, dtype=np.float32)
    truth = np.ascontiguousarray(truth, dtype=np.float32)
    in_maps = [
        {
            "preds": preds[c * BC : (c + 1) * BC],
            "truth": truth[c * BC : (c + 1) * BC],
        }
        for c in range(N_CORES)
    ]
    res = run_bass_kernel_spmd(nc, in_maps, core_ids=list(range(N_CORES)))
    return np.concatenate([res.results[c]["loss"] for c in range(N_CORES)], axis=0)


# revision 13
# speedup vs baseline: 1.1466x; 1.1466x over previous
"""Log-normal NLL loss kernel for Trainium2 (8 NeuronCores, data-parallel).

Reference math (per sample):
    preds = (mu1, log_sigma1, mu2, log_sigma2); y = truth
    s   = exp(2*log_sigma1) + exp(2*log_sigma2)          # sigma1^2 + sigma2^2
    mu  = mu1 + mu2
    out = log(y) + 0.5*log(2*pi*s) + (log(y) - mu)^2 / (2*s)

The division by s is computed on the Scalar (ACT) engine via exp/ln
(v = exp(ln(z^2/2) - ln(2*pi*s) + ln(2*pi))), which avoids the slow DVE
iterative-divide and balances work: 7 ACT passes + 6 DVE passes per
element, both under the HBM roofline (~24 MB/core @ ~358 GB/s ~= 67 us).
"""

import os
import sys

import numpy as np

for _p in ("/opt/trn_rl_repo", os.path.expanduser("~/.axon_site/_ro/trn_rl_repo")):
    if os.path.isdir(_p) and _p not in sys.path:
        sys.path.insert(0, _p)

import concourse.bacc as bacc
import concourse.bass as bass
import concourse.mybir as mybir
import concourse.tile as tile
from concourse.bass_utils import run_bass_kernel_spmd

B = 8388608
N_CORES = 8
BC = B // N_CORES  # 1048576 samples per core
P = 128            # SBUF partitions

LOG_2PI = float(np.log(2.0 * np.pi))
TWO_PI = float(2.0 * np.pi)
INV_SQRT2 = float(1.0 / np.sqrt(2.0))

_ACT_SET = "natural_log_exp_and_others"  # contains exp, ln AND square
_tables_patched = False


def _pin_act_table_set():
    """Pin all activations to one table set.

    The stock insert_act_table_loads maps each function to the FIRST set
    containing it (exp/square -> exp_and_others, ln -> natural_log_exp...),
    which forces a ~1.3us ACT_TABLE_LOAD at nearly every function switch
    (21 loads, ~27us of Scalar-engine time for this kernel).  Exp, Ln and
    Square all live in natural_log_exp_and_others, so present every other
    set as empty; indices (act_func_set_id) are preserved so walrus still
    resolves the right table.
    """
    global _tables_patched
    if _tables_patched:
        return
    import concourse.hw_specs as hw_specs

    real_fn = hw_specs.get_activation_tables

    def pinned(module_arch):
        real = real_fn(module_arch)
        assert _ACT_SET in real, sorted(real)
        return {
            name: (fns if name == _ACT_SET else set())
            for name, fns in real.items()
        }

    bacc.get_activation_tables = pinned
    _tables_patched = True


def _tile_sizes(r: int, f: int) -> list[int]:
    """Tile sizes summing to r: small tiles at BOTH ends (fast pipeline
    fill and drain), full-size `f` tiles in the middle."""
    taper = [f // 4, f // 4, f // 2]
    if r < 2 * sum(taper) + f or r % f != 0:
        g = min(f, r)
        while r % g:
            g //= 2
        return [g] * (r // g)
    mid = r - 2 * sum(taper)
    return taper + [f] * (mid // f) + taper[::-1]


def build_nc(bc: int = BC, f: int = 2048) -> bass.Bass:
    """Build the per-core Bass program for a shard of `bc` samples.

    `f` = samples per partition per full tile. Requires bc % (P*f) == 0.
    """
    r = bc // P           # samples per partition
    assert bc % P == 0 and r % f == 0, (bc, f)
    sizes = _tile_sizes(r, f)
    assert sum(sizes) == r

    f32 = mybir.dt.float32
    bf16 = mybir.dt.bfloat16
    Exp = mybir.ActivationFunctionType.Exp
    Ln = mybir.ActivationFunctionType.Ln
    Square = mybir.ActivationFunctionType.Square
    Alu = mybir.AluOpType

    _pin_act_table_set()
    nc = bacc.Bacc()
    LN_PI = float(np.log(np.pi))

    preds = nc.dram_tensor("preds", [bc, 4], f32, kind="ExternalInput")
    truth = nc.dram_tensor("truth", [bc, 1], f32, kind="ExternalInput")
    loss = nc.dram_tensor("loss", [bc], f32, kind="ExternalOutput")

    # Partition p owns the contiguous sample range [p*r, (p+1)*r) so every
    # DMA moves one contiguous chunk per partition.
    preds_v = preds[:].rearrange("(p r) c -> p r c", p=P)   # [128, r, 4]
    truth_v = truth[:].rearrange("(p r) c -> p (r c)", p=P)  # [128, r]
    loss_v = loss[:].rearrange("(p r) -> p r", p=P)          # [128, r]

    with tile.TileContext(nc) as tc:
        with (
            tc.tile_pool(name="io", bufs=2) as io_pool,
            tc.tile_pool(name="tmp", bufs=2) as tmp,
            tc.tile_pool(name="const", bufs=1) as cpool,
        ):
            lnpi = cpool.tile([P, 1], f32, tag="lnpi", name="lnpi")
            nc.gpsimd.memset(lnpi[:], LN_PI)

            pos = 0
            for ftile in sizes:
                sl = slice(pos, pos + ftile)
                pos += ftile
                pt = io_pool.tile([P, f, 4], f32, tag="preds", name="pt")[
                    :, :ftile, :
                ]
                nc.sync.dma_start(out=pt[:], in_=preds_v[:, sl, :])
                yt = io_pool.tile([P, f], f32, tag="truth", name="yt")[:, :ftile]
                nc.sync.dma_start(out=yt[:], in_=truth_v[:, sl])

                m1 = pt[:, :, 0]
                l1 = pt[:, :, 1]
                m2 = pt[:, :, 2]
                l2 = pt[:, :, 3]

                # Interval-colored scratch tiles; each hosts several
                # short-lived values per iteration (lifetimes disjoint,
                # never in-place).
                ca = tmp.tile([P, f], f32, tag="ca", name="ca")[:, :ftile]
                cb = tmp.tile([P, f], f32, tag="cb", name="cb")[:, :ftile]
                cc = tmp.tile([P, f], f32, tag="cc", name="cc")[:, :ftile]
                cd = tmp.tile([P, f], f32, tag="cd", name="cd")[:, :ftile]
                ce = tmp.tile([P, f], f32, tag="ce", name="ce")[:, :ftile]
                ot = io_pool.tile([P, f], f32, tag="ot", name="ot")[:, :ftile]

                nc.scalar.activation(ca[:], l1, Exp, scale=2.0)    # ca = sigma1^2
                nc.scalar.activation(cb[:], l2, Exp, scale=2.0)    # cb = sigma2^2
                nc.vector.tensor_add(cc[:], ca[:], cb[:])          # cc = s
                nc.scalar.activation(cd[:], yt[:], Ln)             # cd = logy
                nc.vector.tensor_add(ca[:], m1, m2)                # ca = mu
                nc.vector.tensor_sub(cb[:], cd[:], ca[:])          # cb = z = logy-mu
                nc.scalar.activation(ca[:], cb[:], Square)         # ca = z^2
                nc.scalar.activation(ce[:], cc[:], Ln, scale=TWO_PI)  # ce = ln(2pi*s)
                nc.scalar.activation(
                    cc[:], ce[:], Exp, scale=-1.0, bias=lnpi[:, 0:1]
                )  # cc = exp(ln(pi) - ln(2pi*s)) = 1/(2s)
                nc.vector.scalar_tensor_tensor(                    # yt = 0.5*ln(2pi*s)+logy
                    yt[:], ce[:], 0.5, cd[:], Alu.mult, Alu.add
                )
                nc.vector.tensor_mul(cb[:], ca[:], cc[:])          # cb = z^2/(2s)
                nc.vector.tensor_add(ot[:], yt[:], cb[:])          # ot = loss
                nc.sync.dma_start(out=loss_v[:, sl], in_=ot[:])

    nc.compile()
    return nc


_NC = None


def _get_nc() -> bass.Bass:
    global _NC
    if _NC is None:
        _NC = build_nc()
    return _NC


def kernel(preds: np.ndarray, truth: np.ndarray) -> np.ndarray:
    assert preds.shape == (B, 4) and truth.shape == (B, 1)
    nc = _get_nc()
    preds = np.ascontiguousarray(preds, dtype=np.float32)
    truth = np.ascontiguousarray(truth, dtype=np.float32)
    in_maps = [
        {
            "preds": preds[c * BC : (c + 1) * BC],
            "truth": truth[c * BC : (c + 1) * BC],
        }
        for c in range(N_CORES)
    ]
    res = run_bass_kernel_spmd(nc, in_maps, core_ids=list(range(N_CORES)))
    return np.concatenate([res.results[c]["loss"] for c in range(N_CORES)], axis=0)
